# revision 26
# baseline (speedup 1.0000x reference)
"""Trainium2 distributed kernel v2: 4-layer attention encoder (B=4, D=1024, H=16, N=1024).

Sharding: (batch, sequence-half) across 8 NeuronCores — core r owns batch
b = r//2 and sequence half r%2 (512 columns). Per layer each core computes its
K / V^T shard and AllGathers it with its batch peer (2-rank groups).

v2 changes vs v1:
  - fp8(e4m3) DoubleRow matmuls (256-deep contraction per instruction) for the
    Q/K/V projections, attn@V, merge, and the merged-half of p1. Scores, the
    x-half of p1, and p2 stay bf16 (numerics), residual stream fp32.
  - transposed-scores attention: scores^T[keys, q] computed directly, exp'd to
    fp8 on the Scalar engine; attn@V consumes w^T as the moving operand so the
    per-head softmax transpose of v1 (the xbar bottleneck) is gone entirely.
  - softmax row-sums via an all-ones DoubleRow lhsT whose M=128 output is the
    row-sum broadcast across all partitions; normalization is a per-partition
    reciprocal + multiply fused into the PSUM->SBUF drain.
  - p1's x-half is precomputed into SBUF right after the projections, which
    also covers the K/V AllGather latency before attention starts.

Host-side preprocessing (exact, fp32):
  - head-major channel permutation; 1/sqrt(DK) folded into the q drain
  - bk dropped (softmax-invariant); bv folded into the merge bias
  - BatchNorm folded into the p1 relu drain (scale/bias)
  - per-tensor power-of-2 fp8 scales for weights, folded into drain scalars
"""

import numpy as np
import ml_dtypes

import concourse.bass as bass
import concourse.mybir as mybir
import concourse.tile as tile
from concourse import bacc
from concourse.bass_utils import run_bass_kernel_spmd

L, D, H, B, N = 4, 1024, 16, 4, 1024
DK = D // H          # 64
R = 8                # cores
NS = N // 2          # 512 per-core sequence columns
DT = D // 128        # 8 d-tiles
NT = NS // 128       # 4 n-tiles per core
BF = mybir.dt.bfloat16
F32 = mybir.dt.float32
FP8 = mybir.dt.float8e4
BFNP = ml_dtypes.bfloat16
E4 = ml_dtypes.float8_e4m3

KX8 = 4.0    # x -> fp8 scale 2^4
KV8 = 4.0    # v -> fp8 scale 2^4
KM8 = 4.0    # merged -> fp8 scale 2^4

# head-major channel permutation: perm[h*64+dk] = dk*16+h
PERM = np.array([dk * H + h for h in range(H) for dk in range(DK)])


def _kexp(w, target=112.0):
    """Power-of-2 exponent so |w|*2^k <= target."""
    m = float(np.abs(w).max())
    if m == 0.0:
        return 0.0
    return float(np.floor(np.log2(target / m)))


def _wtile_stream(w_t, np_dtype):
    """(C, M) weight -> (M//128, 128, C) [mt, p, ct*128+mo] = w_t[ct*128+p, mt*128+mo].
    Works for both plain bf16 chunks (ct-major) and fp8 DoubleRow pairs
    (pair cp occupies cols cp*256..cp*256+255 with j-stride 128)."""
    c, m = w_t.shape
    a = w_t.reshape(c // 128, 128, m // 128, 128).transpose(2, 1, 0, 3)
    return np.ascontiguousarray(a.reshape(m // 128, 128, -1)).astype(np_dtype)


def _btile(b_vec):
    """(C,) bias -> (128, C//128) [p, ct]."""
    c = b_vec.shape[0]
    return np.ascontiguousarray(b_vec.reshape(c // 128, 128).T).astype(np.float32)


def prepare_host_inputs(inputs):
    Wq, bq = inputs["Wq"], inputs["bq"]
    Wk = inputs["Wk"]
    Wv, bv = inputs["Wv"], inputs["bv"]
    Wm, bm = inputs["Wm"], inputs["bm"]
    Wp1, bp1 = inputs["Wp1"], inputs["bp1"]
    g, beta = inputs["bn_gamma"], inputs["bn_beta"]
    mu, var = inputs["bn_mean"], inputs["bn_var"]
    Wp2 = inputs["Wp2"]

    out = {k: [] for k in ("wq", "wk", "wv", "wm", "wp1m", "wp1x", "wp2",
                           "bq", "mgb", "s1", "b1")}
    scal = {k: [] for k in ("sq", "sk", "sv", "sm")}
    for l in range(L):
        Wq_p = Wq[l][PERM]          # (D out head-major, D in)
        Wk_p = Wk[l][PERM]
        Wv_p = Wv[l][PERM]
        kq = _kexp(Wq_p); kk = _kexp(Wk_p); kv = _kexp(Wv_p)
        out["wq"].append(_wtile_stream((Wq_p * 2.0 ** kq).T, E4))
        out["wk"].append(_wtile_stream((Wk_p * 2.0 ** kk).T, E4))
        # wv resident layout [128, 8192]: [p, ct*1024 + d] = WvT[ct*128+p, d]
        wvt = (Wv_p * 2.0 ** kv).T   # (C in, D out)
        wv_r = wvt.reshape(DT, 128, D).transpose(1, 0, 2).reshape(128, DT * D)
        out["wv"].append(np.ascontiguousarray(wv_r).astype(E4))

        Wm_eff = Wm[l][:, PERM]     # input side head-major
        bm_eff = bm[l] + Wm[l] @ bv[l]
        km = _kexp(Wm_eff)
        out["wm"].append(_wtile_stream((Wm_eff * 2.0 ** km).T, E4))
        out["mgb"].append(_btile(bm_eff * 2.0 ** KM8))

        Wp1m = Wp1[l][:, :D]
        Wp1x = Wp1[l][:, D:]
        kp1 = _kexp(Wp1m)
        out["wp1m"].append(_wtile_stream((Wp1m * 2.0 ** (kp1 - KM8)).T, E4))
        out["wp1x"].append(_wtile_stream((Wp1x * 2.0 ** kp1).T, BFNP))
        out["wp2"].append(_wtile_stream(Wp2[l].T, BFNP))

        out["bq"].append(_btile(bq[l][PERM] / 8.0))
        s1 = g[l] / np.sqrt(var[l] + 1e-5)
        b1 = beta[l] + s1 * (bp1[l] - mu[l])
        out["s1"].append(_btile(s1 * 2.0 ** (-kp1)))
        out["b1"].append(_btile(b1))

        scal["sq"].append(2.0 ** (-kq - KX8) / 8.0)
        scal["sk"].append(2.0 ** (-kk - KX8))
        scal["sv"].append(2.0 ** (KV8 - kv - KX8))
        scal["sm"].append(2.0 ** (KM8 - km - KV8))

    res = {k: np.stack(v) for k, v in out.items()}
    for k in ("bq", "mgb"):
        res[k] = np.ascontiguousarray(res[k].transpose(1, 0, 2).reshape(128, -1))
    for k in ("s1", "b1"):
        res[k] = np.ascontiguousarray(res[k].transpose(1, 0, 2).reshape(128, -1))
    res["ident"] = np.eye(128, dtype=BFNP)
    res["_scal"] = {k: tuple(v) for k, v in scal.items()}
    return res


def shard_x(motion_feats, r):
    b, half = r // 2, r % 2
    m = motion_feats[b, :, half * NS : (half + 1) * NS]
    m = m.reshape(DT, 128, NS).transpose(1, 0, 2)
    return np.ascontiguousarray(m.reshape(128, DT * NS)).astype(np.float32)


def unshard_out(res_list):
    out = np.empty((B, D, N), dtype=np.float32)
    for r, arr in enumerate(res_list):
        b, half = r // 2, r % 2
        m = arr.reshape(128, DT, NS).transpose(1, 0, 2)
        out[b, :, half * NS : (half + 1) * NS] = m.reshape(D, NS)
    return out


def build_nc(scal):
    sq_l, sk_l, sv_l, sm_l = scal["sq"], scal["sk"], scal["sv"], scal["sm"]

    nc = bacc.Bacc("TRN2", target_bir_lowering=False, debug=False, num_devices=R)

    x_in = nc.dram_tensor("x_in", [128, DT * NS], F32, kind="ExternalInput")
    wq_d = nc.dram_tensor("wq", [L, DT, 128, D], FP8, kind="ExternalInput")
    wk_d = nc.dram_tensor("wk", [L, DT, 128, D], FP8, kind="ExternalInput")
    wv_d = nc.dram_tensor("wv", [L, 128, DT * D], FP8, kind="ExternalInput")
    wm_d = nc.dram_tensor("wm", [L, DT, 128, D], FP8, kind="ExternalInput")
    wp1m_d = nc.dram_tensor("wp1m", [L, 16, 128, D], FP8, kind="ExternalInput")
    wp1x_d = nc.dram_tensor("wp1x", [L, 16, 128, D], BF, kind="ExternalInput")
    wp2_d = nc.dram_tensor("wp2", [L, DT, 128, 2 * D], BF, kind="ExternalInput")
    bq_d = nc.dram_tensor("bq", [128, L * 8], F32, kind="ExternalInput")
    mgb_d = nc.dram_tensor("mgb", [128, L * 8], F32, kind="ExternalInput")
    s1_d = nc.dram_tensor("s1", [128, L * 16], F32, kind="ExternalInput")
    b1_d = nc.dram_tensor("b1", [128, L * 16], F32, kind="ExternalInput")
    id_d = nc.dram_tensor("ident", [128, 128], BF, kind="ExternalInput")
    out_e = nc.dram_tensor("out", [128, DT * NS], F32, kind="ExternalOutput")

    ADD = mybir.AluOpType.add
    MUL = mybir.AluOpType.mult
    AF = mybir.ActivationFunctionType
    DR = mybir.MatmulPerfMode.DoubleRow
    GROUPS = [[0, 1], [2, 3], [4, 5], [6, 7]]

    with tile.TileContext(nc) as tc:
        with (
            tc.tile_pool(name="const", bufs=1) as const,
            tc.tile_pool(name="acts", bufs=1) as acts,
            tc.tile_pool(name="wres", bufs=1) as wres,
            tc.tile_pool(name="wstr", bufs=3) as wstr,
            tc.tile_pool(name="wt8", bufs=5) as wt8p,
            tc.tile_pool(name="scl", bufs=3) as sclp,
            tc.tile_pool(name="pp", bufs=2, space="PSUM") as ppp,
            tc.tile_pool(name="av", bufs=1, space="PSUM") as avp,
            tc.tile_pool(name="sc", bufs=2, space="PSUM") as scp,
            tc.tile_pool(name="dram", bufs=2, space="DRAM") as dramp,
        ):
            bq_sb = const.tile([128, L * 8], F32)
            nc.sync.dma_start(bq_sb[:], bq_d[:, :])
            mgb_sb = const.tile([128, L * 8], F32)
            nc.sync.dma_start(mgb_sb[:], mgb_d[:, :])
            s1_sb = const.tile([128, L * 16], F32)
            nc.sync.dma_start(s1_sb[:], s1_d[:, :])
            b1_sb = const.tile([128, L * 16], F32)
            nc.sync.dma_start(b1_sb[:], b1_d[:, :])
            ones8 = const.tile([128, 256], FP8)
            nc.vector.memset(ones8[:], 1.0)
            id_sb = const.tile([128, 128], BF)
            nc.sync.dma_start(id_sb[:], id_d[:, :])

            x_sb = acts.tile([128, DT * NS], F32)
            for q_ in range(4):
                nc.sync.dma_start(
                    x_sb[:, q_ * 1024 : (q_ + 1) * 1024],
                    x_in[:, q_ * 1024 : (q_ + 1) * 1024],
                )
            x_bf = acts.tile([128, DT * NS], BF)
            x_f8 = acts.tile([128, DT * NS], FP8)
            q_bf = acts.tile([128, DT * NS], BF)
            k_sh = acts.tile([128, DT * NS], BF)
            v_sh = acts.tile([128, NT * D], FP8)
            kts = acts.tile([128, DT * N], BF)       # gathered K: [p, t*1024 + m]
            v_all = acts.tile([128, 2 * NT * D], FP8)  # gathered V^T: [p, c*1024 + d]
            attn_f8 = acts.tile([128, DT * NS], FP8)
            mg_f8 = acts.tile([128, DT * NS], FP8)
            h1x = acts.tile([128, 16 * NS], BF)
            h1_bf = acts.tile([128, 16 * NS], BF)

            def xpair(cp):
                return x_f8[:, 2 * cp * NS : (2 * cp + 2) * NS].rearrange(
                    "p (j n) -> p j n", j=2
                )

            def wpair(t, cp):
                return t[:, cp * 256 : (cp + 1) * 256].rearrange(
                    "p (j m) -> p j m", j=2
                )

            def cast_x_chunk(ot):
                nc.vector.tensor_scalar_mul(
                    x_f8[:, ot * NS : (ot + 1) * NS],
                    x_sb[:, ot * NS : (ot + 1) * NS], 2.0 ** KX8,
                )
                nc.vector.tensor_copy(
                    x_bf[:, ot * NS : (ot + 1) * NS],
                    x_sb[:, ot * NS : (ot + 1) * NS],
                )

            for l in range(L):
                if l == 0:
                    for ot in range(DT):
                        cast_x_chunk(ot)

                # ---- K projection (fp8 DR), drains on Scalar ----
                ck_i = dramp.tile([128, DT * NS], BF, tag="cki")
                for mt in range(DT):
                    w_t = wstr.tile([128, D], FP8, tag="wk")
                    nc.sync.dma_start(w_t[:], wk_d[l, mt, :, :])
                    ps = ppp.tile([128, NS], F32, tag="pp")
                    for cp in range(4):
                        nc.tensor.matmul(
                            ps[:], wpair(w_t, cp), xpair(cp),
                            start=(cp == 0), stop=(cp == 3), perf_mode=DR,
                        )
                    nc.scalar.activation(
                        k_sh[:, mt * NS : (mt + 1) * NS], ps[:], AF.Copy,
                        scale=sk_l[l],
                    )
                    nc.sync.dma_start(
                        ck_i[:, mt * NS : (mt + 1) * NS],
                        k_sh[:, mt * NS : (mt + 1) * NS],
                    )
                ck_o = dramp.tile([2 * 128, DT * NS], BF, tag="cko")
                nc.gpsimd.collective_compute(
                    "AllGather", mybir.AluOpType.bypass, replica_groups=GROUPS,
                    ins=[ck_i[:].opt()], outs=[ck_o[:].opt()],
                )

                # ---- V^T projection (fp8 DR) ----
                cv_i = dramp.tile([128, NT * D], FP8, tag="cvi")
                wv_sb = wres.tile([128, DT * D], FP8, tag="wv")
                nc.sync.dma_start(wv_sb[:], wv_d[l, :, :])
                for nt in range(NT):
                    for dh in range(2):
                        ps = ppp.tile([128, NS], F32, tag="pp")
                        for cp in range(4):
                            lhsT = xpair(cp)[:, :, nt * 128 : (nt + 1) * 128]
                            rhs = wv_sb[
                                :, 2 * cp * D : (2 * cp + 2) * D
                            ].rearrange("p (j d) -> p j d", j=2)[
                                :, :, dh * NS : (dh + 1) * NS
                            ]
                            nc.tensor.matmul(
                                ps[:], lhsT, rhs,
                                start=(cp == 0), stop=(cp == 3), perf_mode=DR,
                            )
                        nc.scalar.activation(
                            v_sh[:, nt * D + dh * NS : nt * D + (dh + 1) * NS],
                            ps[:], AF.Copy, scale=sv_l[l],
                        )
                        nc.sync.dma_start(
                            cv_i[:, nt * D + dh * NS : nt * D + (dh + 1) * NS],
                            v_sh[:, nt * D + dh * NS : nt * D + (dh + 1) * NS],
                        )
                cv_o = dramp.tile([2 * 128, NT * D], FP8, tag="cvo")
                nc.gpsimd.collective_compute(
                    "AllGather", mybir.AluOpType.bypass, replica_groups=GROUPS,
                    ins=[cv_i[:].opt()], outs=[cv_o[:].opt()],
                )

                # ---- Q projection (fp8 DR), drain on DVE with bias ----
                for mt in range(DT):
                    w_t = wstr.tile([128, D], FP8, tag="wq")
                    nc.sync.dma_start(w_t[:], wq_d[l, mt, :, :])
                    ps = ppp.tile([128, NS], F32, tag="pp")
                    for cp in range(4):
                        nc.tensor.matmul(
                            ps[:], wpair(w_t, cp), xpair(cp),
                            start=(cp == 0), stop=(cp == 3), perf_mode=DR,
                        )
                    nc.scalar.activation(
                        q_bf[:, mt * NS : (mt + 1) * NS], ps[:], AF.Identity,
                        bias=bq_sb[:, l * 8 + mt : l * 8 + mt + 1], scale=sq_l[l],
                    )

                # ---- p1 x-half tile (bf16): PE filler work ----
                def p1x_mt(mt):
                    w_t = wstr.tile([128, D], BF, tag="wp1x")
                    nc.sync.dma_start(w_t[:], wp1x_d[l, mt, :, :])
                    ps = ppp.tile([128, NS], F32, tag="pp")
                    for ct in range(DT):
                        nc.tensor.matmul(
                            ps[:],
                            w_t[:, ct * 128 : (ct + 1) * 128],
                            x_bf[:, ct * NS : (ct + 1) * NS],
                            start=(ct == 0), stop=(ct == DT - 1),
                        )
                    nc.vector.tensor_copy(h1x[:, mt * NS : (mt + 1) * NS], ps[:])

                # early p1x tiles cover the AllGather latency
                for mt in range(8):
                    p1x_mt(mt)

                # ---- gathered K/V into SBUF ----
                # per-tile gather-in DMAs spread across parallel queues
                ko = ck_o[:].rearrange("(r p) (t m) -> r p t m", r=2, t=DT)
                for t_ in range(DT):
                    nc.sync.dma_start(
                        kts[:, t_ * N : (t_ + 1) * N].rearrange(
                            "p (r m) -> p r m", r=2
                        ),
                        ko[:, :, t_, :].rearrange("r p m -> p r m"),
                    )
                vo = cv_o[:].rearrange("(r p) (nt d) -> r p nt d", r=2, nt=NT)
                for r_ in range(2):
                    nc.sync.dma_start(
                        v_all[
                            :, r_ * NT * D : (r_ + 1) * NT * D
                        ].rearrange("p (nt d) -> p nt d", nt=NT),
                        vo[r_],
                    )

                # ---- attention: one-head-lookahead pipeline ----
                def sc_head(h):
                    t, half = h // 2, (h % 2) * 64
                    wt_t = wt8p.tile([128, 8 * NS], FP8, tag="wt")
                    for cp in range(4):
                        scps = scp.tile([128, 2 * NS], F32, tag="sc")
                        for j in range(2):
                            c = 2 * cp + j
                            nc.tensor.matmul(
                                scps[:, j * NS : (j + 1) * NS],
                                kts[half : half + 64, t * N + c * 128 : t * N + (c + 1) * 128],
                                q_bf[half : half + 64, t * NS : (t + 1) * NS],
                                start=True, stop=True,
                            )
                        nc.scalar.activation(
                            wt_t[:, cp * 2 * NS : (cp + 1) * 2 * NS], scps[:], AF.Exp
                        )
                    return wt_t

                def attnv_mms(t, wt_t, at):
                    for cp in range(4):
                        lhsT = v_all[
                            :, 2 * cp * D : (2 * cp + 2) * D
                        ].rearrange("p (j d) -> p j d", j=2)[
                            :, :, t * 128 : (t + 1) * 128
                        ]
                        nc.tensor.matmul(
                            at[:], lhsT,
                            wt_t[:, cp * 2 * NS : (cp + 1) * 2 * NS].rearrange(
                                "p (j n) -> p j n", j=2
                            ),
                            start=(cp == 0), stop=(cp == 3), perf_mode=DR,
                        )

                def rowsum_dr(wt_t, rsb):
                    # DoubleRow all-ones lhsT: every output row = the row-sum
                    for cp in range(4):
                        nc.tensor.matmul(
                            rsb[:],
                            ones8[:].rearrange("p (j m) -> p j m", j=2),
                            wt_t[:, cp * 2 * NS : (cp + 1) * 2 * NS].rearrange(
                                "p (j n) -> p j n", j=2
                            ),
                            start=(cp == 0), stop=(cp == 3), perf_mode=DR,
                        )

                def cons_tile(t, wt0, wt1):
                    # per-head broadcast row-sums; reciprocal reads the half that
                    # matches the head's attn partitions, right after each stop
                    rcp = sclp.tile([128, NS], F32, tag="rcp")
                    rsb0 = avp.tile([128, NS], F32, tag="rsb")
                    rowsum_dr(wt0, rsb0)
                    nc.vector.reciprocal(rcp[0:64, :], rsb0[0:64, :])
                    rsb1 = avp.tile([128, NS], F32, tag="rsb")
                    rowsum_dr(wt1, rsb1)
                    nc.vector.reciprocal(rcp[64:128, :], rsb1[64:128, :])
                    at0 = avp.tile([128, NS], F32, tag="at")
                    attnv_mms(t, wt0, at0)
                    nc.vector.tensor_tensor(
                        attn_f8[0:64, t * NS : (t + 1) * NS],
                        at0[0:64, :], rcp[0:64, :], op=MUL,
                    )
                    at1 = avp.tile([128, NS], F32, tag="at")
                    attnv_mms(t, wt1, at1)
                    nc.vector.tensor_tensor(
                        attn_f8[64:128, t * NS : (t + 1) * NS],
                        at1[64:128, :], rcp[64:128, :], op=MUL,
                    )

                # pair-wise consume with lookahead; p1x tiles fill the PE
                wts = {}
                for e in range(20):
                    if e >= 4 and e % 2 == 0:
                        t = (e - 4) // 2
                        cons_tile(t, wts.pop(2 * t), wts.pop(2 * t + 1))
                        if 8 + t < 16:
                            p1x_mt(8 + t)
                    if e < 16:
                        wts[e] = sc_head(e)

                # ---- merge (fp8 DR) ----
                for mt in range(DT):
                    w_t = wstr.tile([128, D], FP8, tag="wm")
                    nc.sync.dma_start(w_t[:], wm_d[l, mt, :, :])
                    ps = ppp.tile([128, NS], F32, tag="pp")
                    for cp in range(4):
                        nc.tensor.matmul(
                            ps[:], wpair(w_t, cp),
                            attn_f8[:, 2 * cp * NS : (2 * cp + 2) * NS].rearrange(
                                "p (j n) -> p j n", j=2
                            ),
                            start=(cp == 0), stop=(cp == 3), perf_mode=DR,
                        )
                    nc.scalar.activation(
                        mg_f8[:, mt * NS : (mt + 1) * NS], ps[:], AF.Identity,
                        bias=mgb_sb[:, l * 8 + mt : l * 8 + mt + 1], scale=sm_l[l],
                    )

                # ---- p1 merged-half (fp8 DR) + h1x + BN/relu ----
                for mt in range(16):
                    w_t = wstr.tile([128, D], FP8, tag="wp1m")
                    nc.sync.dma_start(w_t[:], wp1m_d[l, mt, :, :])
                    ps = ppp.tile([128, NS], F32, tag="pp")
                    for cp in range(4):
                        nc.tensor.matmul(
                            ps[:], wpair(w_t, cp),
                            mg_f8[:, 2 * cp * NS : (2 * cp + 2) * NS].rearrange(
                                "p (j n) -> p j n", j=2
                            ),
                            start=(cp == 0), stop=False, perf_mode=DR,
                        )
                    nc.tensor.matmul(
                        ps[:], id_sb[:], h1x[:, mt * NS : (mt + 1) * NS],
                        start=False, stop=True,
                    )
                    nc.scalar.activation(
                        h1_bf[:, mt * NS : (mt + 1) * NS], ps[:], AF.Relu,
                        bias=b1_sb[:, l * 16 + mt : l * 16 + mt + 1],
                        scale=s1_sb[:, l * 16 + mt : l * 16 + mt + 1],
                    )

                # ---- p2 (bf16) + residual ----
                for ot in range(DT):
                    w_t = wstr.tile([128, 2 * D], BF, tag="wp2")
                    nc.sync.dma_start(w_t[:], wp2_d[l, ot, :, :])
                    ps = ppp.tile([128, NS], F32, tag="pp")
                    for ct in range(16):
                        nc.tensor.matmul(
                            ps[:],
                            w_t[:, ct * 128 : (ct + 1) * 128],
                            h1_bf[:, ct * NS : (ct + 1) * NS],
                            start=(ct == 0), stop=(ct == 15),
                        )
                    nc.vector.tensor_tensor(
                        x_sb[:, ot * NS : (ot + 1) * NS],
                        x_sb[:, ot * NS : (ot + 1) * NS],
                        ps[:], op=ADD,
                    )
                    if l < L - 1:
                        cast_x_chunk(ot)

            nc.sync.dma_start(out_e[:, :], x_sb[:])

    nc.finalize()
    return nc


_CACHED = {}


def kernel(**inputs):
    inputs = {k: np.asarray(v) for k, v in inputs.items()}
    host = prepare_host_inputs(inputs)
    scal = host.pop("_scal")

    key = tuple(sorted(scal.items()))
    if _CACHED.get("key") != key:
        _CACHED["nc"] = build_nc(scal)
        _CACHED["key"] = key
    nc = _CACHED["nc"]

    in_maps = build_in_maps(inputs, host)
    res = run_bass_kernel_spmd(nc, in_maps, core_ids=list(range(R)))
    return unshard_out([res.results[r]["out"] for r in range(R)])


def build_in_maps(inputs, host):
    in_maps = []
    for r in range(R):
        m = {
            "x_in": shard_x(inputs["motion_feats"], r),
            "wq": host["wq"], "wk": host["wk"], "wv": host["wv"], "wm": host["wm"],
            "wp1m": host["wp1m"], "wp1x": host["wp1x"], "wp2": host["wp2"],
            "bq": host["bq"], "mgb": host["mgb"], "s1": host["s1"], "b1": host["b1"],
            "ident": host["ident"],
        }
        in_maps.append(m)
    return in_maps
